# revision 23
# baseline (speedup 1.0000x reference)
"""Self-contained Trainium2 Bass kernel for the AttentionBlock problem.

Shapes (hardcoded): x [8, 256, 64, 64] fp32, Wq/Wk [32, 256], bq/bk [32],
Wv [256, 256], bv [256], gamma [1].

Sharding: data-parallel over batch — each of the 8 NeuronCores computes the
full 4096x4096 attention for one batch element. No collectives.

Per-core algorithm (C=256, C8=32, N=4096), fully SBUF-resident, one global
software pipeline over 128 groups of 2 key tiles (16 per 512-query window):
  step ig:  QK[ig]   2 row-tiled K=32 bf16 matmuls -> psum [128, 2, 512]
                     (double-buffered so it overlaps exp[ig-1])
            AV[ig-1] 4 bf16 matmuls accumulate v.T@p into [c, i] PSUM
            exp[ig]  one FD=1024 ACT from PSUM -> pT bf16
            acc += pT (DVE bf16)  - per-partition partial rowsums
  per window: rowsum = ones.T @ acc lanes (2 PE matmuls, broadcast across
  partitions), rinv = gamma * approx-recip(rowsum) (DVE),
  out = av*rinv + x (fp32 exact), epilogue emitted two steps into the NEXT
  window so the PE never stalls on it.  All matmul operands bf16 (fp32
  matmuls are 2-pass LOW_HIGH on trn2); PSUM accumulation stays fp32.
"""

import sys

import numpy as np

if "/opt/trn_rl_repo" not in sys.path:
    sys.path.insert(0, "/opt/trn_rl_repo")

import concourse.bass as bass
import concourse.bacc as bacc
import concourse.tile as tile
from concourse import mybir
from concourse.bass_utils import run_bass_kernel_spmd
from concourse.masks import make_identity

F32 = mybir.dt.float32
BF16 = mybir.dt.bfloat16

C = 256
C8 = 32
P = 128
CH = C // P  # 2 channel chunks


def build_attention_nc(n: int = 4096) -> bass.Bass:
    """Build the single-core Bass program (SPMD across 8 cores)."""
    assert n % 512 == 0 and (n // 128) % 2 == 0
    NT = n // P     # key tiles (j)
    IW = n // 512   # query windows (i)
    NG = NT // 2    # groups of 2 key tiles per window
    NGLOB = IW * NG

    nc = bacc.Bacc("TRN2", target_bir_lowering=False)
    x_d = nc.declare_dram_parameter("x", [C, n], F32, isOutput=False)
    wq_d = nc.declare_dram_parameter("Wq", [C8, C], F32, isOutput=False)
    bq_d = nc.declare_dram_parameter("bq", [C8], F32, isOutput=False)
    wk_d = nc.declare_dram_parameter("Wk", [C8, C], F32, isOutput=False)
    bk_d = nc.declare_dram_parameter("bk", [C8], F32, isOutput=False)
    wv_d = nc.declare_dram_parameter("Wv", [C, C], F32, isOutput=False)
    bv_d = nc.declare_dram_parameter("bv", [C], F32, isOutput=False)
    gamma_d = nc.declare_dram_parameter("gamma", [1], F32, isOutput=False)
    out_d = nc.declare_dram_parameter("out", [C, n], F32, isOutput=True)

    with tile.TileContext(nc) as tc:
        with (
            tc.tile_pool(name="const", bufs=1) as const,
            tc.tile_pool(name="xpool", bufs=1) as xpool,
            tc.tile_pool(name="qkpool", bufs=1) as qkpool,
            tc.tile_pool(name="vtpool", bufs=1) as vtpool,
            tc.tile_pool(name="ptpool", bufs=8) as ptpool,
            tc.tile_pool(name="accpool", bufs=3) as accpool,
            tc.tile_pool(name="smallwork", bufs=4) as smallwork,
            tc.tile_pool(name="outpool", bufs=10) as outpool,
            tc.tile_pool(name="pe_ps", bufs=2, space="PSUM") as pe_ps,
            tc.tile_pool(name="av_ps", bufs=3, space="PSUM") as av_ps,
            tc.tile_pool(name="v_ps", bufs=1, space="PSUM") as v_ps,
        ):
            # ---------------- setup: loads + casts ----------------
            # warm the ACT exp table while DMAs run
            warm_in = const.tile([P, 1], F32, tag="warmin")
            nc.vector.memset(warm_in, 0.0)
            warm_out = const.tile([P, 1], F32, tag="warmout")
            nc.scalar.activation(warm_out, warm_in, mybir.ActivationFunctionType.Exp)

            ident = const.tile([P, P], F32, tag="ident")
            make_identity(nc, ident)

            ones_bf = const.tile([P, P], BF16, tag="ones")
            nc.vector.memset(ones_bf, 1.0)

            # weights/biases go on the GpSimd (SWDGE) queue so they don't sit
            # behind the 16 x-window loads on the sync queue
            bq_sb = const.tile([C8, 1], F32, tag="bq")
            nc.gpsimd.dma_start(out=bq_sb, in_=bq_d[:].rearrange("(p one) -> p one", one=1))
            bk_sb = const.tile([C8, 1], F32, tag="bk")
            nc.gpsimd.dma_start(out=bk_sb, in_=bk_d[:].rearrange("(p one) -> p one", one=1))
            bv_sb = const.tile([P, CH], F32, tag="bv")
            nc.gpsimd.dma_start(
                out=bv_sb, in_=bv_d[:].rearrange("(ch p) -> p ch", p=P)
            )
            gamma_ap = gamma_d[:]
            gamma_sb = const.tile([P, 1], F32, tag="gamma")
            nc.gpsimd.dma_start(
                out=gamma_sb,
                in_=bass.AP(
                    tensor=gamma_ap.tensor, offset=gamma_ap.offset,
                    ap=[[0, P], gamma_ap.ap[0]],
                ),
            )

            gbv = const.tile([P, CH], F32, tag="gbv")
            nc.vector.tensor_scalar_mul(gbv, bv_sb, gamma_sb)

            wq_stage = const.tile([C8, C], F32, tag="wqs")
            nc.gpsimd.dma_start(out=wq_stage, in_=wq_d[:, :])
            wk_stage = const.tile([C8, C], F32, tag="wks")
            nc.gpsimd.dma_start(out=wk_stage, in_=wk_d[:, :])
            wv_stage = const.tile([P, CH, C], F32, tag="wvs")
            nc.gpsimd.dma_start(
                out=wv_stage, in_=wv_d[:, :].rearrange("(a p) c -> p a c", p=P)
            )

            x_w, xb_w = [], []
            for iw in range(IW):
                xt = xpool.tile([P, CH, 512], F32, tag=f"xw{iw}", name=f"xw{iw}")
                for ch in range(CH):
                    nc.sync.dma_start(
                        out=xt[:, ch, :],
                        in_=x_d[ch * P : (ch + 1) * P, bass.ts(iw, 512)],
                    )
                x_w.append(xt)
                xbt = xpool.tile([P, CH, 512], BF16, tag=f"xb{iw}", name=f"xb{iw}")
                nc.vector.tensor_copy(xbt, xt)
                xb_w.append(xbt)

            wqt = const.tile([P, CH, C8], BF16, tag="wqt")  # [c, ch, o] bf16
            wkt = const.tile([P, CH, C8], BF16, tag="wkt")
            for ch in range(CH):
                ps_t = v_ps.tile([P, P], F32, tag="vps", name=f"ps_tq{ch}")
                nc.tensor.transpose(
                    ps_t[:, :C8], wq_stage[:, bass.ts(ch, P)], ident[:C8, :C8]
                )
                nc.vector.tensor_copy(wqt[:, ch, :], ps_t[:, :C8])
                ps_t2 = av_ps.tile([P, P], F32, tag="avps", name=f"ps_tk{ch}")
                nc.tensor.transpose(
                    ps_t2[:, :C8], wk_stage[:, bass.ts(ch, P)], ident[:C8, :C8]
                )
                nc.vector.tensor_copy(wkt[:, ch, :], ps_t2[:, :C8])

            wvt = const.tile([P, CH, C], BF16, tag="wvt")  # [c', ci, o] bf16
            for ci in range(CH):
                for oi in range(CH):
                    pool, ptag = (v_ps, "vps") if oi == 0 else (av_ps, "avps")
                    ps_t3 = pool.tile([P, P], F32, tag=ptag, name=f"ps_tv{ci}{oi}")
                    nc.tensor.transpose(
                        ps_t3, wv_stage[:, oi, bass.ts(ci, P)], ident
                    )
                    nc.vector.tensor_copy(wvt[:, ci, bass.ts(oi, P)], ps_t3)

            # ---------------- projections (bf16 matmuls) ----------------
            # q, k: [32, n] bf16, replicated x2 across partition groups
            q4 = qkpool.tile([2 * C8, n], BF16, tag="q4")
            k4 = qkpool.tile([2 * C8, n], BF16, tag="k4")
            for iw in range(IW):
                win = bass.ts(iw, 512)
                ps_q = av_ps.tile([C8, 512], F32, tag="avps", name=f"ps_q{iw}")
                for ch in range(CH):
                    nc.tensor.matmul(
                        ps_q, wqt[:, ch, :], xb_w[iw][:, ch, :],
                        start=(ch == 0), stop=(ch == CH - 1),
                    )
                nc.scalar.activation(
                    q4[:C8, win], ps_q,
                    mybir.ActivationFunctionType.Identity,
                    bias=bq_sb, scale=1.0,
                )
                ps_k = av_ps.tile([C8, 512], F32, tag="avps", name=f"ps_k{iw}")
                for ch in range(CH):
                    nc.tensor.matmul(
                        ps_k, wkt[:, ch, :], xb_w[iw][:, ch, :],
                        start=(ch == 0), stop=(ch == CH - 1),
                    )
                nc.vector.tensor_scalar_add(k4[:C8, win], ps_k, bk_sb)
            nc.sync.dma_start(out=q4[C8 : 2 * C8, :], in_=q4[:C8, :])
            nc.sync.dma_start(out=k4[C8 : 2 * C8, :], in_=k4[:C8, :])

            # vT per key tile: vt[jt][p, c] = v[c, jt*128+p] + bv[c], bf16.
            # First half emitted up front; second half interleaved into the
            # pipeline so it doesn't head-of-line block the first QK groups.
            vt = [None] * NT

            def emit_vproj(jt, early):
                vtt = vtpool.tile([P, C], BF16, tag=f"vt{jt}", name=f"vt{jt}")
                if early:
                    pool, ptag = pe_ps, "peps"
                else:
                    pool, ptag = (v_ps, "vps") if jt % 2 == 0 else (av_ps, "avps")
                ps_v = pool.tile([P, C], F32, tag=ptag, name=f"ps_v{jt}")
                iww, off = (jt * P) // 512, (jt * P) % 512
                for ch in range(CH):
                    nc.tensor.matmul(
                        ps_v,
                        xb_w[iww][:, ch, off : off + P],
                        wvt[:, ch, :],
                        start=(ch == 0), stop=(ch == CH - 1),
                    )
                nc.vector.tensor_copy(vtt, ps_v)
                vt[jt] = vtt

            n_up = min(16, NT // 2)
            for jt in range(n_up):
                emit_vproj(jt, True)
            vjt_late = list(range(n_up, NT))

            # ---------------- main pipeline ----------------
            state = {}

            def emit_qk_exp_acc(ig):
                iw, g = divmod(ig, NG)
                win = bass.ts(iw, 512)
                if g == 0:
                    state[iw] = {
                        "av": [
                            av_ps.tile([P, 512], F32, tag="avps", name=f"av{i}_{iw}")
                            for i in range(CH)
                        ],
                        "acc": accpool.tile(
                            [P, 2, 512], BF16, tag="acc", name=f"acc_{iw}"
                        ),
                    }
                ps_e = pe_ps.tile([P, 2, 512], F32, tag="peps", name=f"ps_e{ig}")
                for m in range(2):
                    jt = 2 * g + m
                    nc.tensor.matmul(
                        ps_e[:, m, :],
                        k4[m * C8 : (m + 1) * C8, bass.ts(jt, P)],
                        q4[m * C8 : (m + 1) * C8, win],
                        start=True, stop=True,
                        tile_position=(m * C8, 0),
                    )
                pt = ptpool.tile([P, 2, 512], BF16, tag="pt", name=f"pt{ig}")
                nc.scalar.activation(pt, ps_e, mybir.ActivationFunctionType.Exp)
                acc = state[iw]["acc"]
                if g == 0:
                    nc.vector.tensor_copy(acc, pt)
                else:
                    nc.vector.tensor_add(acc, acc, pt)
                return pt

            def emit_av(ig, pt):
                iw, g = divmod(ig, NG)
                av = state[iw]["av"]
                for m in range(2):
                    jt = 2 * g + m
                    for ch in range(CH):
                        nc.tensor.matmul(
                            av[ch],
                            vt[jt][:, bass.ts(ch, P)],
                            pt[:, m, :],
                            start=(g == 0 and m == 0),
                            stop=(g == NG - 1 and m == 1),
                            skip_group_check=True,
                        )
                if g == NG - 1:
                    av_sb = []
                    for ch in range(CH):
                        a_sb = outpool.tile(
                            [P, 512], F32, tag="osb", name=f"avsb{ch}_{iw}"
                        )
                        nc.vector.tensor_copy(a_sb, av[ch])
                        av_sb.append(a_sb)
                    state[iw]["av_sb"] = av_sb

            def emit_epilogue(iw):
                st = state.pop(iw)
                acc, av_sb = st["acc"], st["av_sb"]
                win = bass.ts(iw, 512)
                ps_r = v_ps.tile([P, 512], F32, tag="vps", name=f"ps_r{iw}")
                for m in range(2):
                    nc.tensor.matmul(
                        ps_r, ones_bf, acc[:, m, :],
                        start=(m == 0), stop=(m == 1),
                    )
                rinv = smallwork.tile([P, 512], F32, tag="rinv", name=f"rinv{iw}")
                nc.vector.reciprocal_approx_fast(rinv, ps_r)
                nc.vector.tensor_scalar_mul(rinv, rinv, gamma_sb)
                for ch in range(CH):
                    o_sb = outpool.tile([P, 512], F32, tag="osb", name=f"osb{ch}_{iw}")
                    nc.vector.tensor_mul(o_sb, av_sb[ch], rinv)
                    nc.vector.scalar_tensor_tensor(
                        out=o_sb, in0=o_sb, scalar=gbv[:, ch : ch + 1],
                        in1=x_w[iw][:, ch, :],
                        op0=mybir.AluOpType.add, op1=mybir.AluOpType.add,
                    )
                    nc.sync.dma_start(
                        out=out_d[ch * P : (ch + 1) * P, win], in_=o_sb
                    )

            pts = [None] * (NGLOB + 1)
            for ig in range(NGLOB + 1):
                if ig < NGLOB:
                    pts[ig] = emit_qk_exp_acc(ig)
                    # drip the second half of the v projections into the
                    # first pipeline steps
                    for _ in range(2):
                        if vjt_late:
                            emit_vproj(vjt_late.pop(0), False)
                if ig > 0:
                    emit_av(ig - 1, pts[ig - 1])
                    pts[ig - 1] = None
                if ig >= NG + 4 and (ig - 4) % NG == 0:
                    emit_epilogue((ig - 4) // NG - 1)
            emit_epilogue(IW - 1)

    nc.finalize()
    return nc


_NC_CACHE: dict[int, bass.Bass] = {}


def _get_nc(n: int) -> bass.Bass:
    if n not in _NC_CACHE:
        _NC_CACHE[n] = build_attention_nc(n)
    return _NC_CACHE[n]


def kernel(x, Wq, bq, Wk, bk, Wv, bv, gamma):
    B, c, h, w = x.shape
    n = h * w
    assert B == 8 and c == C
    nc = _get_nc(n)
    xf = np.ascontiguousarray(np.asarray(x, dtype=np.float32).reshape(B, c, n))
    common = {
        "Wq": np.ascontiguousarray(np.asarray(Wq, dtype=np.float32)),
        "bq": np.ascontiguousarray(np.asarray(bq, dtype=np.float32)),
        "Wk": np.ascontiguousarray(np.asarray(Wk, dtype=np.float32)),
        "bk": np.ascontiguousarray(np.asarray(bk, dtype=np.float32)),
        "Wv": np.ascontiguousarray(np.asarray(Wv, dtype=np.float32)),
        "bv": np.ascontiguousarray(np.asarray(bv, dtype=np.float32)),
        "gamma": np.ascontiguousarray(np.asarray(gamma, dtype=np.float32)),
    }
    in_maps = [{"x": xf[b], **common} for b in range(B)]
    res = run_bass_kernel_spmd(nc, in_maps, core_ids=list(range(B)))
    out = np.stack([res.results[b]["out"].reshape(c, h, w) for b in range(B)])
    return out.astype(np.float32)


# revision 24
# speedup vs baseline: 1.1881x; 1.1881x over previous
"""Self-contained Trainium2 Bass kernel for the AttentionBlock problem.

Shapes (hardcoded): x [8, 256, 64, 64] fp32, Wq/Wk [32, 256], bq/bk [32],
Wv [256, 256], bv [256], gamma [1].

Sharding: data-parallel over batch — each of the 8 NeuronCores computes the
full 4096x4096 attention for one batch element. No collectives.

Per-core algorithm (C=256, C8=32, N=4096), fully SBUF-resident, one global
software pipeline over 128 groups of 2 key tiles (16 per 512-query window):
  step ig:  QK[ig]   2 row-tiled K=32 bf16 matmuls -> psum [128, 2, 512]
                     (double-buffered so it overlaps exp[ig-1])
            AV[ig-1] 4 bf16 matmuls accumulate v.T@p into [c, i] PSUM
            exp[ig]  one FD=1024 ACT from PSUM -> pT bf16
            acc += pT (DVE bf16)  - per-partition partial rowsums
  per window: rowsum = ones.T @ acc lanes (2 PE matmuls, broadcast across
  partitions), rinv = gamma * approx-recip(rowsum) (DVE),
  out = av*rinv + x (fp32 exact), epilogue emitted two steps into the NEXT
  window so the PE never stalls on it.  All matmul operands bf16 (fp32
  matmuls are 2-pass LOW_HIGH on trn2); PSUM accumulation stays fp32.
"""

import sys

import numpy as np

if "/opt/trn_rl_repo" not in sys.path:
    sys.path.insert(0, "/opt/trn_rl_repo")

import concourse.bass as bass
import concourse.bacc as bacc
import concourse.tile as tile
from concourse import mybir
from concourse.bass_utils import run_bass_kernel_spmd
from concourse.masks import make_identity

F32 = mybir.dt.float32
BF16 = mybir.dt.bfloat16

C = 256
C8 = 32
P = 128
CH = C // P  # 2 channel chunks


def build_attention_nc(n: int = 4096) -> bass.Bass:
    """Build the single-core Bass program (SPMD across 8 cores)."""
    assert n % 512 == 0 and (n // 128) % 2 == 0
    NT = n // P     # key tiles (j)
    IW = n // 512   # query windows (i)
    NG = NT // 2    # groups of 2 key tiles per window
    NGLOB = IW * NG

    nc = bacc.Bacc("TRN2", target_bir_lowering=False)
    x_d = nc.declare_dram_parameter("x", [C, n], F32, isOutput=False)
    wq_d = nc.declare_dram_parameter("Wq", [C8, C], F32, isOutput=False)
    bq_d = nc.declare_dram_parameter("bq", [C8], F32, isOutput=False)
    wk_d = nc.declare_dram_parameter("Wk", [C8, C], F32, isOutput=False)
    bk_d = nc.declare_dram_parameter("bk", [C8], F32, isOutput=False)
    wv_d = nc.declare_dram_parameter("Wv", [C, C], F32, isOutput=False)
    bv_d = nc.declare_dram_parameter("bv", [C], F32, isOutput=False)
    gamma_d = nc.declare_dram_parameter("gamma", [1], F32, isOutput=False)
    out_d = nc.declare_dram_parameter("out", [C, n], F32, isOutput=True)

    with tile.TileContext(nc) as tc:
        with (
            tc.tile_pool(name="const", bufs=1) as const,
            tc.tile_pool(name="xpool", bufs=1) as xpool,
            tc.tile_pool(name="qkpool", bufs=1) as qkpool,
            tc.tile_pool(name="vtpool", bufs=1) as vtpool,
            tc.tile_pool(name="ptpool", bufs=8) as ptpool,
            tc.tile_pool(name="accpool", bufs=3) as accpool,
            tc.tile_pool(name="smallwork", bufs=4) as smallwork,
            tc.tile_pool(name="outpool", bufs=10) as outpool,
            tc.tile_pool(name="pe_ps", bufs=2, space="PSUM") as pe_ps,
            tc.tile_pool(name="av_ps", bufs=3, space="PSUM") as av_ps,
            tc.tile_pool(name="v_ps", bufs=1, space="PSUM") as v_ps,
        ):
            # ---------------- setup: loads + casts ----------------
            # warm the ACT exp table while DMAs run
            warm_in = const.tile([P, 1], F32, tag="warmin")
            nc.vector.memset(warm_in, 0.0)
            warm_out = const.tile([P, 1], F32, tag="warmout")
            nc.scalar.activation(warm_out, warm_in, mybir.ActivationFunctionType.Exp)

            ident = const.tile([P, P], F32, tag="ident")
            make_identity(nc, ident)

            ones_bf = const.tile([P, P], BF16, tag="ones")
            nc.vector.memset(ones_bf, 1.0)

            # weights/biases go on the GpSimd (SWDGE) queue so they don't sit
            # behind the 16 x-window loads on the sync queue
            bq_sb = const.tile([C8, 1], F32, tag="bq")
            nc.gpsimd.dma_start(out=bq_sb, in_=bq_d[:].rearrange("(p one) -> p one", one=1))
            bk_sb = const.tile([C8, 1], F32, tag="bk")
            nc.gpsimd.dma_start(out=bk_sb, in_=bk_d[:].rearrange("(p one) -> p one", one=1))
            bv_sb = const.tile([P, CH], F32, tag="bv")
            nc.gpsimd.dma_start(
                out=bv_sb, in_=bv_d[:].rearrange("(ch p) -> p ch", p=P)
            )
            gamma_ap = gamma_d[:]
            gamma_sb = const.tile([P, 1], F32, tag="gamma")
            nc.gpsimd.dma_start(
                out=gamma_sb,
                in_=bass.AP(
                    tensor=gamma_ap.tensor, offset=gamma_ap.offset,
                    ap=[[0, P], gamma_ap.ap[0]],
                ),
            )

            gbv = const.tile([P, CH], F32, tag="gbv")
            nc.vector.tensor_scalar_mul(gbv, bv_sb, gamma_sb)

            wq_stage = const.tile([C8, C], F32, tag="wqs")
            nc.gpsimd.dma_start(out=wq_stage, in_=wq_d[:, :])
            wk_stage = const.tile([C8, C], F32, tag="wks")
            nc.gpsimd.dma_start(out=wk_stage, in_=wk_d[:, :])
            wv_stage = const.tile([P, CH, C], F32, tag="wvs")
            nc.gpsimd.dma_start(
                out=wv_stage, in_=wv_d[:, :].rearrange("(a p) c -> p a c", p=P)
            )

            x_w, xb_w = [], []
            for iw in range(IW):
                xt = xpool.tile([P, CH, 512], F32, tag=f"xw{iw}", name=f"xw{iw}")
                for ch in range(CH):
                    nc.sync.dma_start(
                        out=xt[:, ch, :],
                        in_=x_d[ch * P : (ch + 1) * P, bass.ts(iw, 512)],
                    )
                x_w.append(xt)
                xbt = xpool.tile([P, CH, 512], BF16, tag=f"xb{iw}", name=f"xb{iw}")
                nc.vector.tensor_copy(xbt, xt)
                xb_w.append(xbt)

            wqt = const.tile([P, CH, C8], BF16, tag="wqt")  # [c, ch, o] bf16
            wkt = const.tile([P, CH, C8], BF16, tag="wkt")
            for ch in range(CH):
                ps_t = v_ps.tile([P, P], F32, tag="vps", name=f"ps_tq{ch}")
                nc.tensor.transpose(
                    ps_t[:, :C8], wq_stage[:, bass.ts(ch, P)], ident[:C8, :C8]
                )
                nc.vector.tensor_copy(wqt[:, ch, :], ps_t[:, :C8])
                ps_t2 = av_ps.tile([P, P], F32, tag="avps", name=f"ps_tk{ch}")
                nc.tensor.transpose(
                    ps_t2[:, :C8], wk_stage[:, bass.ts(ch, P)], ident[:C8, :C8]
                )
                nc.vector.tensor_copy(wkt[:, ch, :], ps_t2[:, :C8])

            wvt = const.tile([P, CH, C], BF16, tag="wvt")  # [c', ci, o] bf16
            for ci in range(CH):
                for oi in range(CH):
                    pool, ptag = (v_ps, "vps") if oi == 0 else (av_ps, "avps")
                    ps_t3 = pool.tile([P, P], F32, tag=ptag, name=f"ps_tv{ci}{oi}")
                    nc.tensor.transpose(
                        ps_t3, wv_stage[:, oi, bass.ts(ci, P)], ident
                    )
                    nc.vector.tensor_copy(wvt[:, ci, bass.ts(oi, P)], ps_t3)

            # ---------------- projections (bf16 matmuls) ----------------
            # q, k: [32, n] bf16, replicated x2 across partition groups
            q4 = qkpool.tile([2 * C8, n], BF16, tag="q4")
            k4 = qkpool.tile([2 * C8, n], BF16, tag="k4")
            for iw in range(IW):
                win = bass.ts(iw, 512)
                ps_q = av_ps.tile([C8, 512], F32, tag="avps", name=f"ps_q{iw}")
                for ch in range(CH):
                    nc.tensor.matmul(
                        ps_q, wqt[:, ch, :], xb_w[iw][:, ch, :],
                        start=(ch == 0), stop=(ch == CH - 1),
                    )
                nc.scalar.activation(
                    q4[:C8, win], ps_q,
                    mybir.ActivationFunctionType.Identity,
                    bias=bq_sb, scale=1.0,
                )
                ps_k = av_ps.tile([C8, 512], F32, tag="avps", name=f"ps_k{iw}")
                for ch in range(CH):
                    nc.tensor.matmul(
                        ps_k, wkt[:, ch, :], xb_w[iw][:, ch, :],
                        start=(ch == 0), stop=(ch == CH - 1),
                    )
                nc.vector.tensor_scalar_add(k4[:C8, win], ps_k, bk_sb)
            nc.sync.dma_start(out=q4[C8 : 2 * C8, :], in_=q4[:C8, :])
            nc.sync.dma_start(out=k4[C8 : 2 * C8, :], in_=k4[:C8, :])

            # vT per key tile: vt[jt][p, c] = v[c, jt*128+p] + bv[c], bf16.
            # First half emitted up front; second half interleaved into the
            # pipeline so it doesn't head-of-line block the first QK groups.
            vt = [None] * NT

            def emit_vproj(jt, early):
                vtt = vtpool.tile([P, C], BF16, tag=f"vt{jt}", name=f"vt{jt}")
                if early:
                    pool, ptag = pe_ps, "peps"
                else:
                    pool, ptag = (v_ps, "vps") if jt % 2 == 0 else (av_ps, "avps")
                ps_v = pool.tile([P, C], F32, tag=ptag, name=f"ps_v{jt}")
                iww, off = (jt * P) // 512, (jt * P) % 512
                for ch in range(CH):
                    nc.tensor.matmul(
                        ps_v,
                        xb_w[iww][:, ch, off : off + P],
                        wvt[:, ch, :],
                        start=(ch == 0), stop=(ch == CH - 1),
                    )
                nc.vector.tensor_copy(vtt, ps_v)
                vt[jt] = vtt

            n_up = min(16, NT // 2)
            for jt in range(n_up):
                emit_vproj(jt, True)
            vjt_late = list(range(n_up, NT))

            # ---------------- main pipeline ----------------
            state = {}

            def emit_qk_exp_acc(ig):
                iw, g = divmod(ig, NG)
                win = bass.ts(iw, 512)
                if g == 0:
                    state[iw] = {
                        "av": [
                            av_ps.tile([P, 512], F32, tag="avps", name=f"av{i}_{iw}")
                            for i in range(CH)
                        ],
                        "acc": accpool.tile(
                            [P, 2, 512], BF16, tag="acc", name=f"acc_{iw}"
                        ),
                    }
                ps_e = pe_ps.tile([P, 2, 512], F32, tag="peps", name=f"ps_e{ig}")
                for m in range(2):
                    jt = 2 * g + m
                    nc.tensor.matmul(
                        ps_e[:, m, :],
                        k4[m * C8 : (m + 1) * C8, bass.ts(jt, P)],
                        q4[m * C8 : (m + 1) * C8, win],
                        start=True, stop=True,
                        tile_position=(m * C8, 0),
                    )
                pt = ptpool.tile([P, 2, 512], BF16, tag="pt", name=f"pt{ig}")
                nc.scalar.activation(pt, ps_e, mybir.ActivationFunctionType.Exp)
                acc = state[iw]["acc"]
                if g == 0:
                    nc.vector.tensor_copy(acc, pt)
                else:
                    nc.vector.tensor_add(acc, acc, pt)
                return pt

            def emit_av(ig, pt):
                iw, g = divmod(ig, NG)
                av = state[iw]["av"]
                for m in range(2):
                    jt = 2 * g + m
                    for ch in range(CH):
                        nc.tensor.matmul(
                            av[ch],
                            vt[jt][:, bass.ts(ch, P)],
                            pt[:, m, :],
                            start=(g == 0 and m == 0),
                            stop=(g == NG - 1 and m == 1),
                            skip_group_check=True,
                        )
                if g == NG - 1:
                    av_sb = []
                    for ch in range(CH):
                        a_sb = outpool.tile(
                            [P, 512], F32, tag="osb", name=f"avsb{ch}_{iw}"
                        )
                        nc.vector.tensor_copy(a_sb, av[ch])
                        av_sb.append(a_sb)
                    state[iw]["av_sb"] = av_sb

            def emit_epilogue(iw):
                st = state.pop(iw)
                acc, av_sb = st["acc"], st["av_sb"]
                win = bass.ts(iw, 512)
                ps_r = v_ps.tile([P, 512], F32, tag="vps", name=f"ps_r{iw}")
                for m in range(2):
                    nc.tensor.matmul(
                        ps_r, ones_bf, acc[:, m, :],
                        start=(m == 0), stop=(m == 1),
                    )
                rinv = smallwork.tile([P, 512], F32, tag="rinv", name=f"rinv{iw}")
                nc.vector.reciprocal_approx_fast(rinv, ps_r)
                nc.vector.tensor_scalar_mul(rinv, rinv, gamma_sb)
                for ch in range(CH):
                    o_sb = outpool.tile([P, 512], F32, tag="osb", name=f"osb{ch}_{iw}")
                    nc.vector.tensor_mul(o_sb, av_sb[ch], rinv)
                    nc.vector.scalar_tensor_tensor(
                        out=o_sb, in0=o_sb, scalar=gbv[:, ch : ch + 1],
                        in1=x_w[iw][:, ch, :],
                        op0=mybir.AluOpType.add, op1=mybir.AluOpType.add,
                    )
                    nc.sync.dma_start(
                        out=out_d[ch * P : (ch + 1) * P, win], in_=o_sb
                    )

            pts = [None] * (NGLOB + 1)
            for ig in range(NGLOB + 1):
                if ig < NGLOB:
                    pts[ig] = emit_qk_exp_acc(ig)
                    # drip the second half of the v projections into the
                    # first pipeline steps
                    for _ in range(2):
                        if vjt_late:
                            emit_vproj(vjt_late.pop(0), False)
                if ig > 0:
                    emit_av(ig - 1, pts[ig - 1])
                    pts[ig - 1] = None
                if ig >= NG + 2 and (ig - 2) % NG == 0:
                    emit_epilogue((ig - 2) // NG - 1)
            emit_epilogue(IW - 1)

    nc.finalize()
    return nc


_NC_CACHE: dict[int, bass.Bass] = {}


def _get_nc(n: int) -> bass.Bass:
    if n not in _NC_CACHE:
        _NC_CACHE[n] = build_attention_nc(n)
    return _NC_CACHE[n]


def kernel(x, Wq, bq, Wk, bk, Wv, bv, gamma):
    B, c, h, w = x.shape
    n = h * w
    assert B == 8 and c == C
    nc = _get_nc(n)
    xf = np.ascontiguousarray(np.asarray(x, dtype=np.float32).reshape(B, c, n))
    common = {
        "Wq": np.ascontiguousarray(np.asarray(Wq, dtype=np.float32)),
        "bq": np.ascontiguousarray(np.asarray(bq, dtype=np.float32)),
        "Wk": np.ascontiguousarray(np.asarray(Wk, dtype=np.float32)),
        "bk": np.ascontiguousarray(np.asarray(bk, dtype=np.float32)),
        "Wv": np.ascontiguousarray(np.asarray(Wv, dtype=np.float32)),
        "bv": np.ascontiguousarray(np.asarray(bv, dtype=np.float32)),
        "gamma": np.ascontiguousarray(np.asarray(gamma, dtype=np.float32)),
    }
    in_maps = [{"x": xf[b], **common} for b in range(B)]
    res = run_bass_kernel_spmd(nc, in_maps, core_ids=list(range(B)))
    out = np.stack([res.results[b]["out"].reshape(c, h, w) for b in range(B)])
    return out.astype(np.float32)
